# revision 4
# baseline (speedup 1.0000x reference)
"""Deterministic MoE router kernel for Trainium2 (8 NeuronCores, SPMD).

Computes, for hidden_states [4, 4096, 2048] f32 and gate_w [128, 2048] f32:
  router_logits  = hidden @ gate_w.T            [4, 4096, 128] f32
  expert_indices = top-6 (deterministic ties)   [4, 4096, 6]   int32
  expert_weights = softmax(top-6 orig logits)   [4, 4096, 6]   f32

Sharding: data-parallel over tokens (B*S = 16384 -> 2048 tokens/core), the
tiny gate weight is replicated. Each shard is laid out [H, tokens] on the
host so the contraction dim lands on SBUF partitions.

Per-core device program:
  - gate chunks [128h, 128e] are the stationary matmul operand (fp32, exact)
  - hiddenT tiles [128h, 512tok] stream through PE; PSUM accumulates
    logitsT [128e, 512tok] over 16 h-chunks
  - PE transposes logitsT back to [tok, e]; DVE computes adjusted logits
    (tie-breaker subtract), top-8 via max/max_index, softmax of the top-6
    original values (ACT exp with fused accumulation).
"""

import sys

for _p in ("/opt/trn_rl_repo",):
    if _p not in sys.path:
        sys.path.insert(0, _p)

import numpy as np

import concourse.bacc as bacc
import concourse.mybir as mybir
import concourse.tile as tile
from concourse.bass_utils import run_bass_kernel_spmd

F32 = mybir.dt.float32
I32 = mybir.dt.int32
U32 = mybir.dt.uint32

B, S, H, E, K = 4, 4096, 2048, 128, 6
N_CORES = 8
N_TOK = B * S
T = N_TOK // N_CORES            # tokens per core (2048)
NCH = H // 128                  # contraction chunks (16)
GRP = 512                       # tokens per PSUM accumulation group
NG = T // GRP                   # groups per core (4)

_cache = {}


def _build():
    nc = bacc.Bacc("TRN2", target_bir_lowering=False, debug=False)

    hid_t = nc.dram_tensor("hid_t", [H, T], F32, kind="ExternalInput")
    gwt_d = nc.dram_tensor("gwt", [H, E], F32, kind="ExternalInput")
    iden_d = nc.dram_tensor("iden", [128, 128], F32, kind="ExternalInput")
    tie_d = nc.dram_tensor("tie", [128, E], F32, kind="ExternalInput")

    logits_d = nc.dram_tensor("logits", [T, E], F32, kind="ExternalOutput")
    idx_d = nc.dram_tensor("idx", [T, K], I32, kind="ExternalOutput")
    w_d = nc.dram_tensor("w", [T, K], F32, kind="ExternalOutput")

    Exp = mybir.ActivationFunctionType.Exp
    sub = mybir.AluOpType.subtract
    mult = mybir.AluOpType.mult
    add = mybir.AluOpType.add

    with tile.TileContext(nc) as tc:
        with (
            tc.tile_pool(name="const", bufs=1) as cpool,
            tc.tile_pool(name="ht", bufs=NG * NCH) as htpool,
            tc.tile_pool(name="lgT", bufs=3) as lgtpool,
            tc.tile_pool(name="stage", bufs=1) as spool,
            tc.tile_pool(name="work", bufs=4) as wpool,
            tc.tile_pool(name="psmm", bufs=2, space="PSUM") as psmm,
            tc.tile_pool(name="pstr", bufs=4, space="PSUM") as pstr,
            tc.tile_pool(name="psdummy", bufs=1, space="PSUM") as psdummy,
        ):
            gw = cpool.tile([128, NCH * E], F32)
            nc.sync.dma_start(gw[:], gwt_d.ap().rearrange("(c p) e -> p c e", p=128))
            iden = cpool.tile([128, 128], F32)
            nc.sync.dma_start(iden[:], iden_d[:])
            tie = cpool.tile([128, E], F32)
            nc.sync.dma_start(tie[:], tie_d[:])

            # all input DMAs up front, group-major so group 0 lands first;
            # alternate issue between the two HWDGE engines (SP / ACT)
            hts = {}
            for g in range(NG):
                for c in range(NCH):
                    t = htpool.tile([128, GRP], F32, tag="ht", name=f"ht_{g}_{c}")
                    eng = nc.sync if (g * NCH + c) % 2 == 0 else nc.scalar
                    eng.dma_start(
                        t[:],
                        hid_t[c * 128 : (c + 1) * 128, g * GRP : (g + 1) * GRP],
                    )
                    hts[(g, c)] = t

            # warm up the PE clock gate while the first DMAs land
            dummy = cpool.tile([128, GRP], F32)
            nc.vector.memset(dummy[:], 0.0)
            psd = psdummy.tile([128, GRP], F32)
            for _ in range(4):
                nc.tensor.matmul(
                    psd[:], dummy[:, 0:E], dummy[:], start=True, stop=True,
                    skip_group_check=True,
                )

            NT_T = T // 128     # token tiles per core (16)
            lg_stage = spool.tile([128, NT_T * E], F32)
            ix_stage = spool.tile([128, NT_T * K], U32)
            w_stage = spool.tile([128, NT_T * K], F32)

            for g in range(NG):
                ps_acc = psmm.tile([128, GRP], F32, tag="mm", name=f"ps_{g}")
                for c in range(NCH):
                    nc.tensor.matmul(
                        ps_acc[:],
                        gw[:, c * E : (c + 1) * E],
                        hts[(g, c)][:],
                        start=(c == 0),
                        stop=(c == NCH - 1),
                    )

                lgT = lgtpool.tile([128, GRP], F32, tag="lgT")
                nc.vector.tensor_copy(lgT[:], ps_acc[:])

                for s in range(GRP // 128):
                    ti = g * (GRP // 128) + s  # token tile index
                    pt = pstr.tile([128, 128], F32, tag="tr")
                    nc.tensor.transpose(
                        pt[:], lgT[:, s * 128 : (s + 1) * 128], iden[:]
                    )

                    nc.scalar.copy(lg_stage[:, ti * E : (ti + 1) * E], pt[:])

                    adj = wpool.tile([128, E], F32, tag="adj")
                    nc.vector.tensor_tensor(adj[:], pt[:], tie[:], op=sub)

                    mx = wpool.tile([128, 8], F32, tag="mx")
                    ix = wpool.tile([128, 8], U32, tag="ix")
                    nc.vector.max(mx[:], adj[:])
                    nc.vector.max_index(ix[:], mx[:], adj[:])
                    nc.vector.tensor_copy(
                        ix_stage[:, ti * K : (ti + 1) * K], ix[:, 0:K]
                    )

                    # original top-6: adjusted + idx*1e-9 (fp32, matches ref)
                    idxf = wpool.tile([128, K], F32, tag="idxf")
                    nc.vector.tensor_copy(idxf[:], ix[:, 0:K])
                    orig = wpool.tile([128, K], F32, tag="orig")
                    nc.vector.scalar_tensor_tensor(
                        orig[:], idxf[:], float(np.float32(1e-9)), mx[:, 0:K],
                        op0=mult, op1=add,
                    )
                    # softmax over the 6 values (no max-shift: |logit| <~ 8)
                    ex = wpool.tile([128, K], F32, tag="ex")
                    sm = wpool.tile([128, 1], F32, tag="sm")
                    nc.scalar.activation(ex[:], orig[:], Exp, accum_out=sm[:])
                    rc = wpool.tile([128, 1], F32, tag="rc")
                    nc.vector.reciprocal(rc[:], sm[:])
                    nc.vector.tensor_scalar(
                        w_stage[:, ti * K : (ti + 1) * K], ex[:], rc[:], None,
                        op0=mult,
                    )

                # flush logits for this group as soon as its tiles are staged
                nc.sync.dma_start(
                    logits_d.ap()
                    .rearrange("(g t p) e -> g p t e", p=128, g=NG)[g],
                    lg_stage[:, g * 4 * E : (g + 1) * 4 * E],
                )

            nc.sync.dma_start(
                idx_d.ap().rearrange("(t p) k -> p t k", p=128),
                ix_stage[:].bitcast(I32),
            )
            nc.sync.dma_start(
                w_d.ap().rearrange("(t p) k -> p t k", p=128), w_stage[:]
            )

    nc.compile()
    return nc


def _get_nc():
    if "nc" not in _cache:
        _cache["nc"] = _build()
    return _cache["nc"]


def _host_inputs(hidden_states, gate_w):
    flat = np.ascontiguousarray(hidden_states, dtype=np.float32).reshape(N_TOK, H)
    gwt = np.ascontiguousarray(gate_w.T.astype(np.float32, copy=False))
    iden = np.eye(128, dtype=np.float32)
    tie1 = np.arange(E, dtype=np.float32) * np.float32(1e-9)
    tie = np.broadcast_to(tie1, (128, E)).copy()
    in_maps = []
    for i in range(N_CORES):
        shard_t = np.ascontiguousarray(flat[i * T : (i + 1) * T, :].T)
        in_maps.append({"hid_t": shard_t, "gwt": gwt, "iden": iden, "tie": tie})
    return in_maps


def _gather(results):
    logits = np.concatenate([r["logits"] for r in results], axis=0)
    idx = np.concatenate([r["idx"] for r in results], axis=0)
    w = np.concatenate([r["w"] for r in results], axis=0)
    return (
        logits.reshape(B, S, E),
        idx.reshape(B, S, K).astype(np.int32),
        w.reshape(B, S, K).astype(np.float32),
    )


def run(hidden_states, gate_w, trace=False, **trace_kwargs):
    nc = _get_nc()
    in_maps = _host_inputs(hidden_states, gate_w)
    res = run_bass_kernel_spmd(
        nc, in_maps, list(range(N_CORES)), trace=trace, **trace_kwargs
    )
    return _gather(res.results), res


def kernel(hidden_states, gate_w):
    out, _ = run(hidden_states, gate_w)
    return out


# revision 5
# speedup vs baseline: 1.3321x; 1.3321x over previous
"""Deterministic MoE router kernel for Trainium2 (8 NeuronCores, SPMD).

Computes, for hidden_states [4, 4096, 2048] f32 and gate_w [128, 2048] f32:
  router_logits  = hidden @ gate_w.T            [4, 4096, 128] f32
  expert_indices = top-6 (deterministic ties)   [4, 4096, 6]   int32
  expert_weights = softmax(top-6 orig logits)   [4, 4096, 6]   f32

Sharding: data-parallel over tokens (B*S = 16384 -> 2048 tokens/core); the
tiny gate weight is replicated. Each shard is laid out on the host as
[group, h-chunk, 128, 512] so every DMA tile is a fully contiguous 256KB
block with the contraction dim on SBUF partitions.

Per-core device program:
  - gate chunks [128h, 128e] are the stationary matmul operand (fp32: exact,
    2-pass LOW/HIGH at 4 cyc/row)
  - hiddenT tiles [128h, 512tok] stream through PE; PSUM accumulates
    logitsT [128e, 512tok] over 16 h-chunks per 512-token group
  - the PSUM->SBUF copy subtracts the deterministic tie-breaker
    (idx*1e-9, a per-partition scalar in this layout)
  - PE transposes adjusted logits back to [tok, e]; DVE finds top-8 via
    max/max_index, ACT computes exp with fused accumulation for the softmax
  - outputs: adjusted logits [tok, e] (host adds the tie row back) and a
    packed per-tile [8 idx u32 | 6 w f32 | 2 pad] stage with 1KB DMA lines
"""

import sys

for _p in ("/opt/trn_rl_repo",):
    if _p not in sys.path:
        sys.path.insert(0, _p)

import numpy as np

import concourse.bacc as bacc
import concourse.mybir as mybir
import concourse.tile as tile
from concourse.bass_utils import run_bass_kernel_spmd

F32 = mybir.dt.float32
I32 = mybir.dt.int32
U32 = mybir.dt.uint32

B, S, H, E, K = 4, 4096, 2048, 128, 6
N_CORES = 8
N_TOK = B * S
T = N_TOK // N_CORES            # tokens per core (2048)
NCH = H // 128                  # contraction chunks (16)
GRP = 512                       # tokens per PSUM accumulation group
NG = T // GRP                   # groups per core (4)
NT_T = T // 128                 # token tiles per core (16)
PK = 16                         # packed stage stride per tile (8 idx + 6 w + 2)

TIE = np.arange(E, dtype=np.float32) * np.float32(1e-9)

_cache = {}


def _build():
    nc = bacc.Bacc("TRN2", target_bir_lowering=False, debug=False)

    hid_t = nc.dram_tensor("hid_t", [NG, NCH, 128, GRP], F32, kind="ExternalInput")
    gwt_d = nc.dram_tensor("gwt", [H, E], F32, kind="ExternalInput")
    iden_d = nc.dram_tensor("iden", [128, 128], F32, kind="ExternalInput")
    ntie_d = nc.dram_tensor("ntie", [128, 1], F32, kind="ExternalInput")

    adj_d = nc.dram_tensor("adj", [T, E], F32, kind="ExternalOutput")
    iw_d = nc.dram_tensor("iw", [128, NT_T * PK], U32, kind="ExternalOutput")

    Exp = mybir.ActivationFunctionType.Exp
    add = mybir.AluOpType.add
    mult = mybir.AluOpType.mult

    with tile.TileContext(nc) as tc:
        with (
            tc.tile_pool(name="const", bufs=1) as cpool,
            tc.tile_pool(name="ht", bufs=NG * NCH) as htpool,
            tc.tile_pool(name="lgT", bufs=3) as lgtpool,
            tc.tile_pool(name="stage", bufs=1) as spool,
            tc.tile_pool(name="work", bufs=4) as wpool,
            tc.tile_pool(name="psmm", bufs=3, space="PSUM") as psmm,
            tc.tile_pool(name="pstr", bufs=4, space="PSUM") as pstr,
            tc.tile_pool(name="psdummy", bufs=1, space="PSUM") as psdummy,
        ):
            gw = cpool.tile([128, NCH * E], F32)
            nc.sync.dma_start(gw[:], gwt_d.ap().rearrange("(c p) e -> p c e", p=128))
            iden = cpool.tile([128, 128], F32)
            nc.sync.dma_start(iden[:], iden_d[:])
            ntie = cpool.tile([128, 1], F32)
            nc.sync.dma_start(ntie[:], ntie_d[:])

            # all input DMAs up front on SP, group-major: group 0 lands first
            hts = {}
            for g in range(NG):
                for c in range(NCH):
                    t = htpool.tile([128, GRP], F32, tag="ht", name=f"ht_{g}_{c}")
                    nc.sync.dma_start(t[:], hid_t[g, c])
                    hts[(g, c)] = t

            # warm up the PE clock gate while the first DMAs land
            dummy = cpool.tile([128, GRP], F32)
            nc.vector.memset(dummy[:], 0.0)
            psd = psdummy.tile([128, GRP], F32)
            for _ in range(3):
                nc.tensor.matmul(
                    psd[:], dummy[:, 0:E], dummy[:], start=True, stop=True,
                    skip_group_check=True,
                )

            lg_stage = spool.tile([128, NT_T * E], F32)
            iw_stage = spool.tile([128, NT_T * PK], U32)

            for g in range(NG):
                ps_acc = psmm.tile([128, GRP], F32, tag="mm", name=f"ps_{g}")
                for c in range(NCH):
                    nc.tensor.matmul(
                        ps_acc[:],
                        gw[:, c * E : (c + 1) * E],
                        hts[(g, c)][:],
                        start=(c == 0),
                        stop=(c == NCH - 1),
                    )

                # PSUM -> SBUF with fused tie-breaker subtract (per-partition)
                lgT = lgtpool.tile([128, GRP], F32, tag="lgT")
                nc.vector.tensor_scalar(lgT[:], ps_acc[:], ntie[:, 0:1], None, op0=add)

                for s in range(GRP // 128):
                    ti = g * (GRP // 128) + s  # token tile index
                    pt = pstr.tile([128, 128], F32, tag="tr")
                    nc.tensor.transpose(
                        pt[:], lgT[:, s * 128 : (s + 1) * 128], iden[:]
                    )

                    # adjusted logits [tok, e] -> stage (host adds tie back)
                    nc.scalar.copy(lg_stage[:, ti * E : (ti + 1) * E], pt[:])

                    mx = wpool.tile([128, 8], F32, tag="mx")
                    nc.vector.max(mx[:], pt[:])
                    nc.vector.max_index(
                        iw_stage[:, ti * PK : ti * PK + 8], mx[:], pt[:]
                    )

                    # original top-6: adjusted + idx*1e-9 (fp32, matches ref)
                    idxf = wpool.tile([128, K], F32, tag="idxf")
                    nc.vector.tensor_copy(
                        idxf[:], iw_stage[:, ti * PK : ti * PK + K]
                    )
                    orig = wpool.tile([128, K], F32, tag="orig")
                    nc.vector.scalar_tensor_tensor(
                        orig[:], idxf[:], float(np.float32(1e-9)), mx[:, 0:K],
                        op0=mult, op1=add,
                    )
                    # softmax over the 6 values (no max-shift: |logit| <~ 8)
                    ex = wpool.tile([128, K], F32, tag="ex")
                    sm = wpool.tile([128, 1], F32, tag="sm")
                    nc.scalar.activation(ex[:], orig[:], Exp, accum_out=sm[:])
                    rc = wpool.tile([128, 1], F32, tag="rc")
                    nc.vector.reciprocal(rc[:], sm[:])
                    nc.vector.tensor_scalar(
                        iw_stage[:, ti * PK + 8 : ti * PK + 8 + K].bitcast(F32),
                        ex[:], rc[:], None, op0=mult,
                    )

                # flush adjusted logits for this group once staged
                nc.sync.dma_start(
                    adj_d.ap().rearrange("(g t p) e -> g p t e", p=128, g=NG)[g],
                    lg_stage[:, g * 4 * E : (g + 1) * 4 * E],
                )

            nc.sync.dma_start(iw_d[:], iw_stage[:])

    nc.compile()
    return nc


def _get_nc():
    if "nc" not in _cache:
        _cache["nc"] = _build()
    return _cache["nc"]


def _host_inputs(hidden_states, gate_w):
    flat = np.ascontiguousarray(hidden_states, dtype=np.float32).reshape(N_TOK, H)
    gwt = np.ascontiguousarray(gate_w.T.astype(np.float32, copy=False))
    iden = np.eye(128, dtype=np.float32)
    ntie = (-TIE[:128]).reshape(128, 1).copy()
    in_maps = []
    for i in range(N_CORES):
        shard = flat[i * T : (i + 1) * T, :]           # [T, H]
        # -> [NG, NCH, 128, GRP]: shard.T is [H, T]; block both dims
        st = shard.T.reshape(NCH, 128, NG, GRP).transpose(2, 0, 1, 3)
        in_maps.append(
            {
                "hid_t": np.ascontiguousarray(st),
                "gwt": gwt,
                "iden": iden,
                "ntie": ntie,
            }
        )
    return in_maps


def _gather(results):
    adj = np.concatenate([r["adj"] for r in results], axis=0)
    logits = adj + TIE[None, :]
    idx_parts = []
    w_parts = []
    for r in results:
        iw = r["iw"].reshape(128, NT_T, PK)
        idx_parts.append(iw[:, :, 0:K].transpose(1, 0, 2).reshape(T, K))
        w_parts.append(
            iw[:, :, 8 : 8 + K].view(np.float32).transpose(1, 0, 2).reshape(T, K)
        )
    idx = np.concatenate(idx_parts, axis=0)
    w = np.concatenate(w_parts, axis=0)
    return (
        logits.reshape(B, S, E).astype(np.float32),
        idx.reshape(B, S, K).astype(np.int32),
        w.reshape(B, S, K).astype(np.float32),
    )


def run(hidden_states, gate_w, trace=False, **trace_kwargs):
    nc = _get_nc()
    in_maps = _host_inputs(hidden_states, gate_w)
    res = run_bass_kernel_spmd(
        nc, in_maps, list(range(N_CORES)), trace=trace, **trace_kwargs
    )
    return _gather(res.results), res


def kernel(hidden_states, gate_w):
    out, _ = run(hidden_states, gate_w)
    return out
